# revision 23
# baseline (speedup 1.0000x reference)
"""Bidirectional Mamba on 8 Trainium2 NeuronCores.

Sharding: the 8 independent (batch b in 0..3, direction in {fwd, bwd}) units
map 1:1 onto the 8 cores — zero cross-core communication. Each core runs one
full Mamba pass on x[b] (time-flipped on host for the backward direction):

  per core:  xT [512, 2048]  ->  out [512, 2048]   (d_model x time, transposed)

Host glue (numpy): transpose/flip inputs, pre-transpose weights, fold
A = -exp(A_log), final  y[b] = out_f.T + flip(out_b.T).

Kernel structure per core (T=2048, d_inner=1024 as 8 blocks of 128 channels):
  A: GEMM1 (x @ W_in) for xi and z on PE; depthwise causal conv-4 as four
     diagonal-matmul PSUM accumulations on PE; bias+silu fused on ACT.
  B: x_dbl GEMM, dt GEMM (+softplus on ACT).
  C: per (t-chunk 512, state s 0..15, dblock 0..7):
       dA = exp(A[d,s] * dt)      one ACT op (per-partition scale = A[:, s])
       b  = u * B_s               DVE fp16 (u = dt*conv_out, B_s row-broadcast)
       h  = tensor_tensor_scan(dA, b)   the T-sequential scan, fp32 state
       prod = h * C_s             DVE fp16
       y_psum += I @ prod         PE identity-matmul accumulation over s
     y initialized with diag(D) @ xc (skip term), gated by silu(z) on DVE,
     then out = W_out @ y_gated on PE.
"""

import sys

for _p in ("/opt/trn_rl_repo", "/root/.axon_site/_ro/trn_rl_repo"):
    if _p not in sys.path:
        sys.path.append(_p)

import numpy as np

import concourse.bacc as bacc
import concourse.bass as bass
import concourse.tile as tile
from concourse import mybir
from concourse.masks import make_identity
from concourse import library_config

F16 = mybir.dt.float16
F32 = mybir.dt.float32
AF = mybir.ActivationFunctionType
ALU = mybir.AluOpType

T = 2048          # sequence length per core
C = 512           # d_model
D = 1024          # d_inner
S = 16            # d_state
R = 32            # dt_rank
NDB = D // 128    # 8 d-blocks
TCH = 512         # t-chunk
NTC = T // TCH    # 4
KC = C // 128     # 4 k-chunks for the d_model contraction


def build_nc() -> bass.Bass:
    nc = bacc.Bacc("TRN2", target_bir_lowering=False, debug=False)

    xT = nc.dram_tensor("xT", [C, T], F16, kind="ExternalInput")
    w_in_xT = nc.dram_tensor("w_in_xT", [C, D], F16, kind="ExternalInput")
    w_in_zT = nc.dram_tensor("w_in_zT", [C, D], F16, kind="ExternalInput")
    conv_w = nc.dram_tensor("conv_w", [D, 4], F32, kind="ExternalInput")
    conv_b = nc.dram_tensor("conv_b", [D, 1], F32, kind="ExternalInput")
    w_xT = nc.dram_tensor("w_xT", [D, R + 2 * S], F16, kind="ExternalInput")
    w_dtT = nc.dram_tensor("w_dtT", [R, D], F16, kind="ExternalInput")
    b_dt = nc.dram_tensor("b_dt", [D, 1], F32, kind="ExternalInput")
    A_t = nc.dram_tensor("A", [D, S], F32, kind="ExternalInput")
    Dp = nc.dram_tensor("Dp", [D, 1], F32, kind="ExternalInput")
    w_outT = nc.dram_tensor("w_outT", [D, C], F32 if False else F16, kind="ExternalInput")
    out = nc.dram_tensor("out", [C, T], F32, kind="ExternalOutput")

    with tile.TileContext(nc) as tc:
        with tc.tile_pool(name="persist", bufs=1) as pp:
            # ---- persistent SBUF tensors ----
            sb_xc = pp.tile([128, NDB, T], F16)     # silu(conv(xi))
            sb_zs = pp.tile([128, NDB, T], F16)     # silu(z)
            sb_wout = pp.tile([128, NDB, C], F16)
            sb_wx = pp.tile([128, NDB, R + 2 * S], F16)
            sb_wdtT = pp.tile([R, D], F16)
            sb_convb = pp.tile([128, NDB, 1], F32)
            sb_bdt = pp.tile([128, NDB, 1], F32)
            sb_A = pp.tile([128, NDB, S], F32)
            sb_Dp = pp.tile([128, NDB, 1], F32)
            sb_I = pp.tile([128, 128], F16)
            sb_diagD = pp.tile([128, NDB, 128], F16)
            sb_xdbl16 = pp.tile([R + 2 * S, T], F16)
            sb_hcar = pp.tile([128, NDB * S], F16)

            nc.sync.dma_start(sb_wout, w_outT.rearrange("(n p) c -> p n c", p=128))
            nc.sync.dma_start(sb_wx, w_xT.rearrange("(n p) c -> p n c", p=128))
            nc.sync.dma_start(sb_wdtT, w_dtT[:, :])
            nc.sync.dma_start(sb_convb, conv_b.rearrange("(n p) c -> p n c", p=128))
            nc.sync.dma_start(sb_bdt, b_dt.rearrange("(n p) c -> p n c", p=128))
            nc.sync.dma_start(sb_A, A_t.rearrange("(n p) c -> p n c", p=128))
            nc.sync.dma_start(sb_Dp, Dp.rearrange("(n p) c -> p n c", p=128))
            make_identity(nc, sb_I)
            for db in range(NDB):
                nc.vector.tensor_scalar_mul(
                    sb_diagD[:, db, :], sb_I, sb_Dp[:, db, :]
                )

            # ================= Phase A: GEMM1 + conv + silu =================
            with (
                tc.tile_pool(name="pha", bufs=1) as pa,
                tc.tile_pool(name="pha_w", bufs=2) as paw,
                tc.tile_pool(name="pha_psum", bufs=2, space="PSUM") as pap,
            ):
                sb_xT = pa.tile([128, KC, T], F16)
                sb_winx = pa.tile([128, KC, D], F16)
                sb_winz = pa.tile([128, KC, D], F16)
                sb_convw = pa.tile([128, NDB, 4], F32)
                nc.sync.dma_start(sb_xT, xT.rearrange("(n p) t -> p n t", p=128))
                nc.sync.dma_start(sb_winx, w_in_xT.rearrange("(n p) d -> p n d", p=128))
                nc.sync.dma_start(sb_winz, w_in_zT.rearrange("(n p) d -> p n d", p=128))
                nc.sync.dma_start(sb_convw, conv_w.rearrange("(n p) k -> p n k", p=128))

                for db in range(NDB):
                    dsl = slice(db * 128, (db + 1) * 128)
                    # conv tap diagonal matrices for this d-block
                    dg = paw.tile([128, 4, 128], F16, tag="dg")
                    for k in range(4):
                        nc.vector.tensor_scalar_mul(
                            dg[:, k, :], sb_I, sb_convw[:, db, k : k + 1]
                        )
                    # xi_pre with 3-col zero left pad
                    xip = paw.tile([128, T + 3], F16, tag="xip")
                    nc.vector.memset(xip[:, 0:3], 0.0)
                    for tc_i in range(NTC):
                        tsl = slice(tc_i * TCH, (tc_i + 1) * TCH)
                        ps = pap.tile([128, TCH], F32, tag="mm")
                        for k in range(KC):
                            nc.tensor.matmul(
                                ps,
                                sb_winx[:, k, dsl],
                                sb_xT[:, k, tsl],
                                start=(k == 0),
                                stop=(k == KC - 1),
                            )
                        nc.scalar.copy(xip[:, 3 + tc_i * TCH : 3 + (tc_i + 1) * TCH], ps)
                        # z GEMM + silu (= z * sigmoid(z); CoreSim has no Silu)
                        pz = pap.tile([128, TCH], F32, tag="mmz")
                        for k in range(KC):
                            nc.tensor.matmul(
                                pz,
                                sb_winz[:, k, dsl],
                                sb_xT[:, k, tsl],
                                start=(k == 0),
                                stop=(k == KC - 1),
                            )
                        sg = paw.tile([128, TCH], F16, tag="sg")
                        nc.scalar.activation(sg, pz, AF.Sigmoid)
                        nc.vector.tensor_mul(sb_zs[:, db, tsl], pz, sg)
                    # depthwise conv via 4 diagonal-matmul accumulations
                    for tc_i in range(NTC):
                        tsl = slice(tc_i * TCH, (tc_i + 1) * TCH)
                        pc = pap.tile([128, TCH], F32, tag="mmc")
                        for k in range(4):
                            nc.tensor.matmul(
                                pc,
                                dg[:, k, :],
                                xip[:, k + tc_i * TCH : k + tc_i * TCH + TCH],
                                start=(k == 0),
                                stop=(k == 3),
                            )
                        cg = paw.tile([128, TCH], F16, tag="cg")
                        nc.scalar.activation(
                            cg, pc, AF.Sigmoid, bias=sb_convb[:, db, :]
                        )
                        cv = paw.tile([128, TCH], F16, tag="cv")
                        nc.scalar.activation(
                            cv, pc, AF.Identity, bias=sb_convb[:, db, :]
                        )
                        nc.vector.tensor_mul(sb_xc[:, db, tsl], cv, cg)

            # ================= Phase B: x_dbl, dt =================
            dram_bc = nc.dram_tensor("xdbl_bc", [2 * S, T], F16, kind="Internal")
            with tc.tile_pool(name="phb_psum", bufs=2, space="PSUM") as pbp:
                for tc_i in range(NTC):
                    tsl = slice(tc_i * TCH, (tc_i + 1) * TCH)
                    pxd = pbp.tile([R + 2 * S, TCH], F32, tag="xd")
                    for db in range(NDB):
                        nc.tensor.matmul(
                            pxd,
                            sb_wx[:, db, :],
                            sb_xc[:, db, tsl],
                            start=(db == 0),
                            stop=(db == NDB - 1),
                        )
                    nc.vector.tensor_copy(sb_xdbl16[:, tsl], pxd)
                # bounce B/C rows to DRAM so they can be partition-broadcast
                nc.sync.dma_start(dram_bc[:, :], sb_xdbl16[R : R + 2 * S, :])

            # ================= Phase C: scan + output =================
            with (
                tc.tile_pool(name="bc", bufs=1) as pbc,
                tc.tile_pool(name="work", bufs=6) as pw,
                tc.tile_pool(name="dtp", bufs=1) as pdt_pool,
                tc.tile_pool(name="yg", bufs=2) as pyg,
                tc.tile_pool(name="ypsum", bufs=1, space="PSUM") as pyp,
                tc.tile_pool(name="opsum", bufs=2, space="PSUM") as pop,
                tc.tile_pool(name="dpsum", bufs=2, space="PSUM") as pdp,
            ):
                for tc_i in range(NTC):
                    tsl = slice(tc_i * TCH, (tc_i + 1) * TCH)
                    # dt = softplus(W_dt @ x_dbl[:R] + b_dt), u = dt * xc
                    dts, us = [], []
                    for db in range(NDB):
                        dsl = slice(db * 128, (db + 1) * 128)
                        pdt = pdp.tile([128, TCH], F32, tag="dt")
                        nc.tensor.matmul(pdt, sb_wdtT[:, dsl], sb_xdbl16[0:R, tsl])
                        # softplus = ln(1 + exp(v)); CoreSim has no Softplus
                        ev = pw.tile([128, TCH], F32, tag="ev")
                        nc.scalar.activation(ev, pdt, AF.Exp, bias=sb_bdt[:, db, :])
                        dtt = pdt_pool.tile([128, TCH], F16, tag=f"dt{db}")
                        nc.scalar.activation(dtt, ev, AF.Ln, bias=1.0)
                        dts.append(dtt)
                        u = pdt_pool.tile([128, TCH], F16, tag=f"u{db}")
                        nc.vector.tensor_mul(u, dtt, sb_xc[:, db, tsl])
                        us.append(u)
                    # B/C rows broadcast across partitions, all 16 states
                    bball = pbc.tile([128, S, TCH], F16, tag="bball")
                    cball = pbc.tile([128, S, TCH], F16, tag="cball")
                    # one DMA each: partition-step-0 broadcast x 16 state rows
                    brows = dram_bc[0:S, tsl]
                    crows = dram_bc[S : 2 * S, tsl]
                    bc_src = bass.AP(
                        tensor=brows.tensor,
                        offset=brows.offset,
                        ap=[[0, 128]] + [list(d) for d in brows.ap],
                    )
                    cc_src = bass.AP(
                        tensor=crows.tensor,
                        offset=crows.offset,
                        ap=[[0, 128]] + [list(d) for d in crows.ap],
                    )
                    nc.sync.dma_start(bball, bc_src)
                    nc.sync.dma_start(cball, cc_src)
                    ygs = []
                    for grp in range(2):
                        gdbs = range(grp * 4, grp * 4 + 4)
                        pys = {}
                        for db in gdbs:
                            py = pyp.tile([128, TCH], F32, tag=f"y{db % 4}")
                            nc.tensor.matmul(
                                py, sb_diagD[:, db, :], sb_xc[:, db, tsl],
                                start=True, stop=False, skip_group_check=True,
                            )
                            pys[db] = py
                        for s in range(S):
                            for db in gdbs:
                                ci = db * S + s
                                dA = pw.tile([128, TCH], F16, tag="dA")
                                nc.scalar.activation(
                                    dA, dts[db], AF.Exp,
                                    scale=sb_A[:, db, s : s + 1],
                                )
                                bt = pw.tile([128, TCH], F16, tag="bt")
                                nc.vector.tensor_mul(bt, us[db], bball[:, s, :])
                                h = pw.tile([128, TCH], F16, tag="h")
                                init = 0.0 if tc_i == 0 else sb_hcar[:, ci : ci + 1]
                                nc.vector.tensor_tensor_scan(
                                    h, dA, bt, init, op0=ALU.mult, op1=ALU.add
                                )
                                if tc_i < NTC - 1:
                                    nc.vector.tensor_copy(
                                        sb_hcar[:, ci : ci + 1], h[:, TCH - 1 : TCH]
                                    )
                                prod = pw.tile([128, TCH], F16, tag="prod")
                                nc.vector.tensor_mul(prod, h, cball[:, s, :])
                                nc.tensor.matmul(
                                    pys[db], sb_I, prod,
                                    start=False, stop=(s == S - 1),
                                    skip_group_check=True,
                                )
                        # gate with silu(z)
                        for db in gdbs:
                            yg = pyg.tile([128, TCH], F16, tag=f"yg{db}")
                            nc.vector.tensor_mul(yg, pys[db], sb_zs[:, db, tsl])
                            ygs.append(yg)
                    for m in range(C // 128):
                        po = pop.tile([128, TCH], F32, tag="o")
                        for db in range(NDB):
                            nc.tensor.matmul(
                                po,
                                sb_wout[:, db, m * 128 : (m + 1) * 128],
                                ygs[db],
                                start=(db == 0),
                                stop=(db == NDB - 1),
                            )
                        ot = pyg.tile([128, TCH], F32, tag="ot")
                        nc.scalar.copy(ot, po)
                        nc.sync.dma_start(out[m * 128 : (m + 1) * 128, tsl], ot)
    nc.finalize()
    return nc


_NC_CACHE = None


def _get_nc():
    global _NC_CACHE
    if _NC_CACHE is None:
        _NC_CACHE = build_nc()
    return _NC_CACHE


def _direction_arrays(inputs, tag):
    W_in = np.asarray(inputs[f"W_in_{tag}"], np.float32)     # [2D, C]
    return {
        "w_in_xT": np.ascontiguousarray(W_in[:D].T).astype(np.float16),
        "w_in_zT": np.ascontiguousarray(W_in[D:].T).astype(np.float16),
        "conv_w": np.asarray(inputs[f"conv_w_{tag}"], np.float32).reshape(D, 4),
        "conv_b": np.asarray(inputs[f"conv_b_{tag}"], np.float32).reshape(D, 1),
        "w_xT": np.ascontiguousarray(
            np.asarray(inputs[f"W_x_{tag}"], np.float32).T
        ).astype(np.float16),
        "w_dtT": np.ascontiguousarray(
            np.asarray(inputs[f"W_dt_{tag}"], np.float32).T
        ).astype(np.float16),
        "b_dt": np.asarray(inputs[f"b_dt_{tag}"], np.float32).reshape(D, 1),
        "A": (-np.exp(np.asarray(inputs[f"A_log_{tag}"], np.float32))),
        "Dp": np.asarray(inputs[f"D_{tag}"], np.float32).reshape(D, 1),
        "w_outT": np.ascontiguousarray(
            np.asarray(inputs[f"W_out_{tag}"], np.float32).T
        ).astype(np.float16),
    }


def make_in_maps(inputs):
    x = np.asarray(inputs["x"], np.float32)  # [4, 2048, 512]
    dirs = {"f": _direction_arrays(inputs, "f"), "b": _direction_arrays(inputs, "b")}
    in_maps = []
    for core in range(8):
        b, tag = core % 4, ("f" if core < 4 else "b")
        xb = x[b] if tag == "f" else x[b][::-1]
        m = dict(dirs[tag])
        m["xT"] = np.ascontiguousarray(xb.T).astype(np.float16)
        in_maps.append(m)
    return in_maps


def assemble_output(outs):
    y = np.empty((4, T, C), np.float32)
    for b in range(4):
        y[b] = outs[b].T + outs[4 + b].T[::-1]
    return y


def kernel(**inputs):
    from concourse import bass_utils

    nc = _get_nc()
    in_maps = make_in_maps(inputs)
    res = bass_utils.run_bass_kernel_spmd(nc, in_maps, core_ids=list(range(8)))
    return assemble_output([r["out"] for r in res.results])


if __name__ == "__main__":
    import reference

    inputs = {k: np.asarray(v) for k, v in reference.setup_inputs().items()}
    got = kernel(**inputs)
    exp = np.asarray(reference.reference(**inputs))
    err = np.abs(got - exp).max() / np.abs(exp).max()
    print("Relative error:", err)
